# revision 19
# baseline (speedup 1.0000x reference)
"""Causal self-attention (B=2, S=2048, D=1024, H=16) on 8 trn2 NeuronCores.

Sharding: core c handles batch b = c // 4 and head-group g = c % 4 (4 heads,
256 feature columns).  QKV weights are column-sharded, the output projection
is row-sharded (Megatron style); the host sums the 4 partial outputs per
batch and adds the (wv_b @ wo_w + wo_b) correction vector.

Device-side layout (per core), bf16 matmul operands, fp32 psum accumulate:
  xT        [128, 8, 2048]   x[b].T, host pre-transposed (d on partitions)
  qT/kT     2 x [128, 2048]  per fs half: rows = local features (2 heads)
  v_all     [128, 16, 260]   per s-tile: 4 heads x (64 v columns + ones col)
  logits^T  psum [128, 1024] two k-tiles of one q-block; exp'd on ACT
  av^T      psum [65, 512]   rows 0-63 = unnormalized out^T, row 64 = denom
  avT       2 x [128, 2048]  normalized attention output, transposed
  out       [2048, 1024]     partial (pre-reduction) output, fp32

Phases are interleaved per s-chunk (projections for chunk i+1 overlap the
attention of q-block i) and share one 8-bank PSUM budget:
  tag A (proj + bcast + out-proj) = 2 banks, lg = 4 banks, av = 2 banks.
"""

import os

import ml_dtypes
import numpy as np

import concourse.bass as bass
import concourse.mybir as mybir
import concourse.tile as tile
from concourse import bacc
from concourse.bass_utils import run_bass_kernel_spmd

F32 = mybir.dt.float32
F32R = mybir.dt.float32r
BF16 = mybir.dt.bfloat16
AF = mybir.ActivationFunctionType

B, S, D = 2, 2048, 1024
H, DH = 16, 64          # heads, head depth
G = 4                   # head groups (cores per batch)
HPG = H // G            # heads per group = 4
F = HPG * DH            # local feature columns = 256
KC = D // 128           # contraction chunks = 8
ST = S // 128           # seq tiles of 128 = 16
QB = S // 512           # q blocks of 512 = 4
SCALE = 1.0 / float(np.sqrt(DH))


def _build(allones: bool):
    nc = bacc.Bacc("TRN2", target_bir_lowering=False, debug=False)

    xT_d = nc.dram_tensor("xT", [KC, 4, 128, 512], BF16, kind="ExternalInput")
    wq_d = nc.dram_tensor("wq", [KC, 128, F], BF16, kind="ExternalInput")
    wk_d = nc.dram_tensor("wk", [KC, 128, F], BF16, kind="ExternalInput")
    wv_d = nc.dram_tensor("wv", [KC, 128, F], BF16, kind="ExternalInput")
    wo_d = nc.dram_tensor("wo", [2, 128, D], BF16, kind="ExternalInput")
    bq_d = nc.dram_tensor("bq", [128, 2], F32, kind="ExternalInput")
    bk_d = nc.dram_tensor("bk", [128, 2], F32, kind="ExternalInput")
    out_d = nc.dram_tensor("out", [S, D], F32, kind="ExternalOutput")
    dbg = bool(int(os.environ.get("KDBG", "0")))
    if dbg:
        dbg_qT = nc.dram_tensor("dbg_qT", [2, 128, S], BF16, kind="ExternalOutput")
        dbg_kT = nc.dram_tensor("dbg_kT", [2, 128, S], BF16, kind="ExternalOutput")
        dbg_v = nc.dram_tensor("dbg_v", [128, ST, HPG * 65], BF16, kind="ExternalOutput")
        dbg_avT = nc.dram_tensor("dbg_avT", [2, 128, S], BF16, kind="ExternalOutput")
    if not allones:
        pad_d = nc.dram_tensor("pad", [1, S], BF16, kind="ExternalInput")

    with tile.TileContext(nc) as tc:
        with (
            tc.tile_pool(name="singles", bufs=1) as singles,
            tc.tile_pool(name="expp", bufs=4) as expp,
            tc.tile_pool(name="recipp", bufs=2) as recipp,
            tc.tile_pool(name="bcsbp", bufs=3) as bcsbp,
            tc.tile_pool(name="outsbp", bufs=3) as outsbp,
            tc.tile_pool(name="psum", bufs=2, space="PSUM") as psum,
        ):
            xT = singles.tile([128, KC, S], BF16, tag="xT")
            wq = singles.tile([128, KC, F], BF16, tag="wq")
            wk = singles.tile([128, KC, F], BF16, tag="wk")
            wv = singles.tile([128, KC, F], BF16, tag="wv")
            wo = singles.tile([128, 2, D], BF16, tag="wo")
            qT = [singles.tile([128, S], BF16, tag=f"qT{i}", name=f"qT{i}") for i in range(2)]
            kT = [singles.tile([128, S], BF16, tag=f"kT{i}", name=f"kT{i}") for i in range(2)]
            v_all = singles.tile([128, ST, HPG * 65], BF16, tag="v_all")
            avT = [singles.tile([128, S], BF16, tag=f"avT{i}", name=f"avT{i}") for i in range(2)]
            bq = singles.tile([128, 2], F32, tag="bq")
            bk = singles.tile([128, 2], F32, tag="bk")

            # --- constant / input DMAs ---
            nc.sync.dma_start(out=wq, in_=wq_d.ap().rearrange("c p f -> p c f"))
            nc.sync.dma_start(out=wk, in_=wk_d.ap().rearrange("c p f -> p c f"))
            for sch in range(4):
                for kc in range(KC):
                    nc.sync.dma_start(
                        out=xT[:, kc, bass.ds(sch * 512, 512)],
                        in_=xT_d.ap()[kc, sch],
                    )
            nc.sync.dma_start(out=wv, in_=wv_d.ap().rearrange("c p f -> p c f"))
            nc.sync.dma_start(out=wo, in_=wo_d.ap().rearrange("c p e -> p c e"))
            nc.sync.dma_start(out=bq, in_=bq_d.ap())
            nc.sync.dma_start(out=bk, in_=bk_d.ap())
            for h in range(HPG):
                nc.gpsimd.memset(v_all[:, :, h * 65 + 64 : h * 65 + 65], 1.0)
            if not allones:
                pad_sb = singles.tile([1, S], BF16, tag="pad")
                ones512 = singles.tile([1, 512], BF16, tag="ones512")
                nc.sync.dma_start(out=pad_sb, in_=pad_d.ap())
                nc.vector.memset(ones512, 1.0)

            def emit_qk(sch, fs):
                ssl = bass.ds(sch * 512, 512)
                fsl = bass.ds(fs * 128, 128)
                for w_sb, b_sb, dst in ((wq, bq, qT), (wk, bk, kT)):
                    ps = psum.tile([128, 512], F32, tag="small", bufs=4, name="ps_qk")
                    for kc in range(KC):
                        nc.tensor.matmul(
                            ps,
                            lhsT=w_sb[:, kc, fsl],
                            rhs=xT[:, kc, ssl],
                            start=(kc == 0),
                            stop=(kc == KC - 1),
                        )
                    nc.vector.tensor_scalar_add(
                        dst[fs][:, ssl], ps, b_sb[:, fs : fs + 1]
                    )

            def emit_v(sch, sts=None):
                for st in sts if sts is not None else range(4 * sch, 4 * sch + 4):
                    vps = psum.tile([128, F], F32, tag="small", bufs=4, name="ps_v")
                    for kc in range(KC):
                        nc.tensor.matmul(
                            vps,
                            lhsT=xT[:, kc, bass.ds(st * 128, 128)],
                            rhs=wv[:, kc, :],
                            start=(kc == 0),
                            stop=(kc == KC - 1),
                        )
                    for h in range(HPG):
                        nc.vector.tensor_copy(
                            v_all[:, st, h * 65 : h * 65 + 64],
                            vps[:, h * 64 : (h + 1) * 64],
                        )

            def emit_proj(sch):
                emit_qk(sch, 0)
                emit_v(sch)
                emit_qk(sch, 1)

            def emit_attention_pair(qb, fs):
                """Both heads (hh=0,1) of fs-half: QK emitted as adjacent
                row-tiled matmuls (array rows 0-63 / 64-127) into separate lg
                tensors so they dispatch concurrently on the PE."""
                qsl = bass.ds(qb * 512, 512)
                nkt = 4 * qb + 4
                avs = [
                    psum.tile([65, 512], F32, tag="small", bufs=4, name=f"ps_av{hh}")
                    for hh in range(2)
                ]
                for j in range((nkt + 1) // 2):
                    kts = [kt for kt in (2 * j, 2 * j + 1) if kt < nkt]
                    lgs = [
                        psum.tile([128, 1024], F32, tag="lg", name=f"ps_lg{hh}")
                        for hh in range(2)
                    ]
                    for i, kt in enumerate(kts):
                        osl = bass.ds(i * 512, 512)
                        for hh in range(2):
                            hsl = bass.ds(hh * 64, 64)
                            if not allones:
                                nc.tensor.matmul(
                                    lgs[hh][:, osl],
                                    lhsT=pad_sb[:, bass.ds(kt * 128, 128)],
                                    rhs=ones512[:, :],
                                    start=True,
                                    stop=False,
                                )
                            nc.tensor.matmul(
                                lgs[hh][:, osl],
                                lhsT=kT[fs][hsl, bass.ds(kt * 128, 128)],
                                rhs=qT[fs][hsl, qsl],
                                start=allones,
                                stop=True,
                            )
                    exs = []
                    for hh in range(2):
                        ex = expp.tile([128, 1024], BF16, tag="ex", name=f"ex{hh}")
                        nc.scalar.activation(
                            ex[:, : 512 * len(kts)],
                            lgs[hh][:, : 512 * len(kts)],
                            AF.Exp,
                            scale=SCALE,
                        )
                        exs.append(ex)
                    for i, kt in enumerate(kts):
                        osl = bass.ds(i * 512, 512)
                        for hh in range(2):
                            h = fs * 2 + hh
                            if kt >= 4 * qb:  # diagonal tile: causal mask
                                nc.gpsimd.affine_select(
                                    out=exs[hh][:, osl],
                                    in_=exs[hh][:, osl],
                                    compare_op=mybir.AluOpType.is_ge,
                                    fill=0.0,
                                    base=qb * 512 - kt * 128,
                                    channel_multiplier=-1,
                                    pattern=[[1, 512]],
                                )
                            nc.tensor.matmul(
                                avs[hh],
                                lhsT=v_all[:, kt, h * 65 : (h + 1) * 65],
                                rhs=exs[hh][:, osl],
                                start=(kt == 0),
                                stop=(kt == nkt - 1),
                            )
                # normalize: row 64 of av holds the softmax denominator
                for hh in range(2):
                    hsl = bass.ds(hh * 64, 64)
                    av = avs[hh]
                    den = recipp.tile([1, 512], F32, tag="den", name="den")
                    nc.vector.tensor_copy(den, av[64:65, :])
                    rf = recipp.tile([1, 512], F32, tag="rf", name="rf")
                    nc.vector.reciprocal_approx_fast(rf, den)
                    bcast = bcsbp.tile([64, 512], F32, tag="bcast", name="bcast")
                    nc.gpsimd.partition_broadcast(bcast, rf)
                    nc.vector.tensor_mul(avT[fs][hsl, qsl], av[0:64, :], bcast)

            def emit_oproj(qb):
                for st in range(4 * qb, 4 * qb + 4):
                    ob = outsbp.tile([128, D], F32, tag="ob", name="ob")
                    for eh in range(2):
                        op = psum.tile([128, 512], F32, tag="small", bufs=4, name="ps_op")
                        for fs in range(2):
                            nc.tensor.matmul(
                                op,
                                lhsT=avT[fs][:, bass.ds(st * 128, 128)],
                                rhs=wo[:, fs, bass.ds(eh * 512, 512)],
                                start=(fs == 0),
                                stop=(fs == 1),
                            )
                        nc.vector.tensor_copy(ob[:, bass.ds(eh * 512, 512)], op)
                    nc.sync.dma_start(out=out_d.ap()[bass.ds(st * 128, 128)], in_=ob)

            with nc.named_scope("proj0"):
                emit_proj(0)
            for qb in range(QB):
                nsch = qb + 1
                for fs in range(2):
                    with nc.named_scope(f"attn{qb}f{fs}"):
                        emit_attention_pair(qb, fs)
                    if nsch < QB:
                        with nc.named_scope(f"proj{nsch}p{fs}"):
                            if fs == 0:
                                emit_qk(nsch, 0)
                                emit_v(nsch, [4 * nsch, 4 * nsch + 1])
                            else:
                                emit_v(nsch, [4 * nsch + 2, 4 * nsch + 3])
                                emit_qk(nsch, 1)
                with nc.named_scope(f"oproj{qb}"):
                    emit_oproj(qb)

            if dbg:
                for i in range(2):
                    nc.sync.dma_start(out=dbg_qT.ap()[i], in_=qT[i])
                    nc.sync.dma_start(out=dbg_kT.ap()[i], in_=kT[i])
                    nc.sync.dma_start(out=dbg_avT.ap()[i], in_=avT[i])
                nc.sync.dma_start(out=dbg_v.ap(), in_=v_all)

    nc.compile()
    return nc


_CACHE: dict = {}


def kernel(
    x,
    padding_mask,
    wq_w,
    wq_b,
    wk_w,
    wk_b,
    wv_w,
    wv_b,
    wo_w,
    wo_b,
    **trace_kwargs,
):
    x = np.asarray(x, dtype=np.float32)
    padding_mask = np.asarray(padding_mask, dtype=np.float32)
    wq_w = np.asarray(wq_w, dtype=np.float32)
    wk_w = np.asarray(wk_w, dtype=np.float32)
    wv_w = np.asarray(wv_w, dtype=np.float32)
    wo_w = np.asarray(wo_w, dtype=np.float32)
    wq_b = np.asarray(wq_b, dtype=np.float32)
    wk_b = np.asarray(wk_b, dtype=np.float32)
    wv_b = np.asarray(wv_b, dtype=np.float32)
    wo_b = np.asarray(wo_b, dtype=np.float32)

    allones = bool(np.all(padding_mask == 1.0))
    if allones not in _CACHE:
        _CACHE[allones] = _build(allones)
    nc = _CACHE[allones]

    bf = ml_dtypes.bfloat16
    in_maps = []
    for c in range(8):
        b, g = c // 4, c % 4
        fsl = slice(g * F, (g + 1) * F)
        xTb = x[b].T.astype(bf)  # (1024, 2048)
        m = {
            "xT": np.ascontiguousarray(
                xTb.reshape(KC, 128, 4, 512).transpose(0, 2, 1, 3)
            ),
            "wq": np.ascontiguousarray(wq_w[:, fsl].astype(bf).reshape(KC, 128, F)),
            "wk": np.ascontiguousarray(wk_w[:, fsl].astype(bf).reshape(KC, 128, F)),
            "wv": np.ascontiguousarray(wv_w[:, fsl].astype(bf).reshape(KC, 128, F)),
            "wo": np.ascontiguousarray(wo_w[fsl, :].astype(bf).reshape(2, 128, D)),
            "bq": np.ascontiguousarray(wq_b[fsl].reshape(2, 128).T),
            "bk": np.ascontiguousarray(wk_b[fsl].reshape(2, 128).T),
        }
        if not allones:
            m["pad"] = ((padding_mask[b] - 1.0) * 8e9).reshape(1, S).astype(bf)
        in_maps.append(m)

    res = run_bass_kernel_spmd(nc, in_maps, core_ids=list(range(8)), **trace_kwargs)

    # host-side reduction over head groups + bias correction
    correction = (wv_b @ wo_w + wo_b).astype(np.float32)
    out = np.empty((B, S, D), dtype=np.float32)
    for b in range(B):
        acc = res.results[4 * b]["out"].copy()
        for g in range(1, G):
            acc += res.results[4 * b + g]["out"]
        out[b] = acc + correction
    kernel._last_results = res
    return out
